# revision 1
# baseline (speedup 1.0000x reference)
"""Fused ARFlow kernel for Trainium2 (8 NeuronCores, data-parallel over batch).

Reference computes three causal K=3 convs (64->256->256->128 ch) with NO
nonlinearity between them, then z = exp(alpha*tanh(ls)+beta)*x + tt.
The convs are linear, so they compose on the host into a single causal K=7
conv (64->128 ch) with an effective bias, exact for t>=4; an x-independent
(weights-only) correction D fixes outputs t<4 where the reference's
zero-padding of *biased* intermediates differs from the composition.

Device kernel per core (4 samples, processed as 2 sample-PAIRS so every
post-matmul op runs at the full 128 partitions):
  - x is loaded per sample as a [128, TS+8] fp16 tile with tap-pair packing
    (partitions 0-63 = x shifted 8, 64-127 = x shifted 7; the second sample
    of a pair uses the SWAPPED layout so its data sits in partitions 64-127
    wherever the pair-packed elementwise ops need it),
  - the K=7 conv is 4 fp16 matmuls of contraction 128 per 512-col chunk;
    outputs are split by weight columns into a "log_s" PSUM tile and a "t"
    PSUM tile, each [128, TS] holding BOTH samples of the pair (64-col
    matmuls auto-col-tile into PE column groups 0/64, which run
    concurrently),
  - ScalarE: TH = tanh(ps_ls + b_ls) [128,TS]; E = exp(TH*alpha+beta) fp16,
  - VectorE: ZM halves = E * x (fp16, 2x mode); ZT = (ps_t + b_t) + ZM,
  - z stored as fp16 (upcast to fp32 on host; well within tolerance).

DMA plan: x2a loads on sync (HWDGE), x2b loads + z stores on gpsimd
(SWDGE) so the two dispatchers run in parallel; constants are merged into
3 DMAs; tile-0 zero halos are memset, and the first pair's x tiles are
issued before the constants so the first matmul starts early.
"""

import numpy as np

import concourse.bacc as bacc
import concourse.bass as bass
import concourse.mybir as mybir
import concourse.tile as tile
from concourse.bass_utils import run_bass_kernel_spmd

N_CORES = 8
B, C, T = 32, 64, 8192
NS = B // N_CORES          # samples per core
TS = 1024                  # time-tile width (multiple of 512)
O = 128                    # output channels (2C)

F32 = mybir.dt.float32
F16 = mybir.dt.float16


# ---------------------------------------------------------------- host math

def _compose(w_in, b_in, w_mid, b_mid, w_out, b_out):
    """W_eff (128, 64, 7), b_eff (128,), D (128, 4)."""
    w_in = np.asarray(w_in, np.float64)
    w_mid = np.asarray(w_mid, np.float64)
    w_out = np.asarray(w_out, np.float64)
    b_in = np.asarray(b_in, np.float64)
    b_mid = np.asarray(b_mid, np.float64)
    b_out = np.asarray(b_out, np.float64)
    H = w_in.shape[0]

    w12 = np.zeros((w_mid.shape[0], w_in.shape[1], 5))
    for i in range(3):
        for j in range(3):
            w12[:, :, i + j] += w_mid[:, :, j] @ w_in[:, :, i]
    w_eff = np.zeros((w_out.shape[0], w_in.shape[1], 7))
    for i in range(5):
        for l in range(3):
            w_eff[:, :, i + l] += w_out[:, :, l] @ w12[:, :, i]

    b_eff = w_out.sum(2) @ (w_mid.sum(2) @ b_in + b_mid) + b_out

    # boundary correction: reference chain on x=0, minus steady-state b_eff
    Tz = 12
    rs1 = np.zeros((H, Tz)) + b_in[:, None]
    rs2 = np.zeros((w_mid.shape[0], Tz))
    for t in range(Tz):
        acc = b_mid.copy()
        for j in range(3):
            tau = t - 2 + j
            if tau >= 0:
                acc = acc + w_mid[:, :, j] @ rs1[:, tau]
        rs2[:, t] = acc
    rs3 = np.zeros((w_out.shape[0], Tz))
    for t in range(Tz):
        acc = b_out.copy()
        for l in range(3):
            tau = t - 2 + l
            if tau >= 0:
                acc = acc + w_out[:, :, l] @ rs2[:, tau]
        rs3[:, t] = acc
    D = rs3[:, 0:4] - b_eff[:, None]
    return (w_eff.astype(np.float32), b_eff.astype(np.float32),
            D.astype(np.float32))


def _pack_weights(w_eff):
    """Stationary lhsT tiles as (p, m, o): m 0-3 = normal layout
    (rows 0-63 = tap 2m, 64-127 = tap 2m+1), m 4-7 = swapped halves."""
    WT = np.zeros((8, 128, 128), np.float32)
    for m in range(4):
        WT[m, 0:64, :] = w_eff[:, :, 2 * m].T
        WT[4 + m, 64:128, :] = w_eff[:, :, 2 * m].T
        if 2 * m + 1 <= 6:
            WT[m, 64:128, :] = w_eff[:, :, 2 * m + 1].T
            WT[4 + m, 0:64, :] = w_eff[:, :, 2 * m + 1].T
    return np.ascontiguousarray(WT.transpose(1, 0, 2))


# ------------------------------------------------------------- device build

def build_nc(ns=NS, t_len=T, ts=TS):
    assert ts % 512 == 0 and t_len % ts == 0
    nc = bacc.Bacc("TRN2", target_bir_lowering=False, debug=False,
                   num_devices=N_CORES)
    xs = nc.dram_tensor("xs", (ns, C, t_len), F16, kind="ExternalInput").ap()
    # host-pretransposed to (p, m, o) so the load is a dense 256 KB DMA
    wt = nc.dram_tensor("wt", (128, 8, 128), F16, kind="ExternalInput").ap()
    # cmat rows: 0-127 = D_ls one-hot lhsT cols, 128-255 = D_t, 256-767 = bind
    cmat = nc.dram_tensor("cmat", (4, 768), F16, kind="ExternalInput").ap()
    # cvec cols: 0 = b_ls, 1 = b_t, 2 = alpha, 3 = beta (dup-packed halves)
    cvec = nc.dram_tensor("cvec", (128, 4), F32, kind="ExternalInput").ap()
    z = nc.dram_tensor("z", (ns, C, t_len), F16, kind="ExternalOutput").ap()

    W = ts + 8
    n_tiles = t_len // ts
    Tanh = mybir.ActivationFunctionType.Tanh
    Exp = mybir.ActivationFunctionType.Exp
    ADD = mybir.AluOpType.add
    MUL = mybir.AluOpType.mult

    with tile.TileContext(nc) as tc:
        with (
            tc.tile_pool(name="consts", bufs=1) as consts,
            tc.tile_pool(name="data", bufs=4) as data,
            tc.tile_pool(name="outs", bufs=3) as outs,
            tc.tile_pool(name="psum", bufs=2, space="PSUM") as psum_pool,
        ):
            def load_x2(x2, s, it, t0, swapped, dma):
                # lo rows get x shifted 8, hi rows x shifted 7; a swapped
                # tile exchanges which partition half holds which shift.
                lo = x2[64:128, :] if swapped else x2[0:64, :]
                hi = x2[0:64, :] if swapped else x2[64:128, :]
                if it == 0:
                    nc.gpsimd.memset(lo[:, 0:8], 0.0)
                    nc.gpsimd.memset(hi[:, 0:7], 0.0)
                    dma(lo[:, 8:W], xs[s, :, 0:ts])
                    dma(hi[:, 7:W], xs[s, :, 0:ts + 1])
                elif it == n_tiles - 1:
                    dma(lo[:, 0:W], xs[s, :, t0 - 8:t0 + ts])
                    dma(hi[:, 0:W - 1], xs[s, :, t0 - 7:t_len])
                else:
                    dma(lo[:, 0:W], xs[s, :, t0 - 8:t0 + ts])
                    dma(hi[:, 0:W], xs[s, :, t0 - 7:t0 + ts + 1])

            def make_x2(pr, it, t0):
                x2a = data.tile([128, W], F16, tag="x2a", name="x2a")
                x2b = data.tile([128, W], F16, tag="x2b", name="x2b")
                load_x2(x2a, 2 * pr, it, t0, False, nc.sync.dma_start)
                load_x2(x2b, 2 * pr + 1, it, t0, True, nc.gpsimd.dma_start)
                return x2a, x2b

            # weights first (the first matmul's only const dependency),
            # then the first pair of x tiles, on both DMA dispatchers
            wt_sb = consts.tile([128, 8, 128], F16)
            nc.sync.dma_start(wt_sb, wt)
            cm_sb = consts.tile([4, 768], F16)
            nc.gpsimd.dma_start(cm_sb, cmat)
            pre = [make_x2(0, 0, 0), make_x2(1, 0, 0)]
            cv_sb = consts.tile([128, 4], F32)
            nc.sync.dma_start(cv_sb, cvec)
            pre.append(make_x2(0, 1, ts))

            # PE warm-up: ~4us of throwaway matmuls during the head's
            # DMA wait flips the HAM clock gate to 8/8 before real work
            warm = data.tile([128, 512], F16, tag="warm", name="warm",
                             bufs=1)
            nc.vector.memset(warm, 0.0)
            warm_ps = psum_pool.tile([128, 512], F32, tag="psls",
                                     name="warm_ps")
            for _ in range(18):
                nc.tensor.matmul(warm_ps[:, 0:512], warm[:, 0:128],
                                 warm[:, 0:512], start=True, stop=True,
                                 skip_group_check=True)
            bls_sb = cv_sb[:, 0:1]
            btt_sb = cv_sb[:, 1:2]
            avec_sb = cv_sb[:, 2:3]
            bevec_sb = cv_sb[:, 3:4]
            dls_sb = cm_sb[:, 0:128]
            dtt_sb = cm_sb[:, 128:256]
            bind_sb = cm_sb[:, 256:768]

            tiles = [(it, pr) for it in range(n_tiles)
                     for pr in range(ns // 2)]
            xq = dict(enumerate(pre))
            pending = None
            for k, (it, pr) in enumerate(tiles):
                t0 = it * ts
                s0, s1 = 2 * pr, 2 * pr + 1
                if True:
                    # stores for the previous tile: their stt has finished
                    # by now, so they never stall the HWDGE FIFOs
                    if pending is not None:
                        pzt, pz0, pz1, pt0 = pending
                        nc.gpsimd.dma_start(z[pz0, :, pt0:pt0 + ts],
                                            pzt[0:64, :])
                        nc.gpsimd.dma_start(z[pz1, :, pt0:pt0 + ts],
                                            pzt[64:128, :])
                        pending = None
                    # prefetch x tiles three ahead
                    kf = k + 3
                    if kf < len(tiles) and kf not in xq:
                        itf, prf = tiles[kf]
                        xq[kf] = make_x2(prf, itf, itf * ts)
                    x2a, x2b = xq.pop(k)

                    ps_ls = psum_pool.tile([128, ts], F32, tag="psls",
                                           name="ps_ls")
                    ps_t = psum_pool.tile([128, ts], F32, tag="pst",
                                          name="ps_t")
                    # conv matmuls: 64-col groups 0 / 64 run concurrently
                    for (ps, o0) in ((ps_ls, 0), (ps_t, 64)):
                        corr = (it == 0)
                        for m in range(4):
                            for h in range(ts // 512):
                                ho = h * 512
                                c0 = ho + 2 * m + 1
                                stop = (m == 3) and not (corr and h == 0)
                                nc.tensor.matmul(
                                    ps[0:64, ho:ho + 512],
                                    wt_sb[:, m, o0:o0 + 64],
                                    x2a[:, c0:c0 + 512],
                                    start=(m == 0), stop=stop)
                                nc.tensor.matmul(
                                    ps[64:128, ho:ho + 512],
                                    wt_sb[:, 4 + m, o0:o0 + 64],
                                    x2b[:, c0:c0 + 512],
                                    start=(m == 0), stop=stop)
                        if corr:
                            nc.tensor.matmul(
                                ps[:, 0:512],
                                dls_sb if o0 == 0 else dtt_sb,
                                bind_sb, start=False, stop=True,
                                skip_group_check=True)

                    # evacuate ps_t to SBUF immediately (cheap DVE op with
                    # the bias folded in) so the PSUM bank frees ~4us
                    # earlier than the end of the elementwise chain
                    tc = outs.tile([128, ts], F16, tag="tc", name="tc")
                    nc.vector.tensor_scalar_add(tc, ps_t, btt_sb)
                    th = outs.tile([128, ts], F16, tag="th", name="th")
                    nc.scalar.activation(th, ps_ls, Tanh, bias=bls_sb)
                    e = outs.tile([128, ts], F16, tag="e", name="e")
                    nc.scalar.activation(e, th, Exp,
                                         bias=bevec_sb, scale=avec_sb)
                    zm = outs.tile([128, ts], F16, tag="zm", name="zm")
                    nc.vector.tensor_tensor(
                        zm[0:64, :], e[0:64, :], x2a[0:64, 8:8 + ts], MUL)
                    nc.vector.tensor_tensor(
                        zm[64:128, :], e[64:128, :], x2b[64:128, 8:8 + ts],
                        MUL)
                    zt = outs.tile([128, ts], F16, tag="zt", name="zt")
                    nc.vector.tensor_tensor(zt, tc, zm, ADD)
                    pending = (zt, s0, s1, t0)

            pzt, pz0, pz1, pt0 = pending
            nc.gpsimd.dma_start(z[pz0, :, pt0:pt0 + ts], pzt[0:64, :])
            nc.gpsimd.dma_start(z[pz1, :, pt0:pt0 + ts], pzt[64:128, :])

    nc.compile()
    return nc


def make_in_maps(x, w_in, b_in, w_mid, b_mid, w_out, b_out, alpha, beta,
                 n_cores=N_CORES):
    w_eff, b_eff, D = _compose(w_in, b_in, w_mid, b_mid, w_out, b_out)
    WT = _pack_weights(w_eff)
    CMAT = np.zeros((4, 768), np.float32)
    CMAT[:, 0:64] = D[0:64, :].T
    CMAT[:, 64:128] = D[0:64, :].T
    CMAT[:, 128:192] = D[64:128, :].T
    CMAT[:, 192:256] = D[64:128, :].T
    for i in range(4):
        CMAT[i, 256 + i] = 1.0
    al = np.asarray(alpha, np.float32).reshape(64)
    be = np.asarray(beta, np.float32).reshape(64)
    CVEC = np.stack([
        np.concatenate([b_eff[0:64], b_eff[0:64]]),
        np.concatenate([b_eff[64:128], b_eff[64:128]]),
        np.concatenate([al, al]),
        np.concatenate([be, be]),
    ], axis=1).astype(np.float32)
    x16 = np.ascontiguousarray(np.asarray(x, np.float32)).astype(np.float16)
    ns = x16.shape[0] // n_cores
    maps = []
    for i in range(n_cores):
        m = dict(xs=np.ascontiguousarray(x16[i * ns:(i + 1) * ns]),
                 wt=WT.astype(np.float16),
                 cmat=CMAT.astype(np.float16),
                 cvec=np.ascontiguousarray(CVEC))
        maps.append(m)
    return maps


_NC_CACHE = {}


def _get_nc():
    if "nc" not in _NC_CACHE:
        _NC_CACHE["nc"] = build_nc()
    return _NC_CACHE["nc"]


def kernel(x, w_in, b_in, w_mid, b_mid, w_out, b_out, alpha, beta,
           _trace=False, _trace_kwargs=None):
    nc = _get_nc()
    in_maps = make_in_maps(x, w_in, b_in, w_mid, b_mid, w_out, b_out,
                           alpha, beta)
    res = run_bass_kernel_spmd(nc, in_maps, core_ids=list(range(N_CORES)),
                               trace=_trace, **(_trace_kwargs or {}))
    out = np.concatenate([r["z"] for r in res.results], axis=0)
    kernel.last_results = res
    return out.astype(np.float32)



# revision 2
# speedup vs baseline: 1.0783x; 1.0783x over previous
"""Fused ARFlow kernel for Trainium2 (8 NeuronCores, data-parallel over batch).

Reference computes three causal K=3 convs (64->256->256->128 ch) with NO
nonlinearity between them, then z = exp(alpha*tanh(ls)+beta)*x + tt.
The convs are linear, so they compose on the host into a single causal K=7
conv (64->128 ch) with an effective bias, exact for t>=4; an x-independent
(weights-only) correction D fixes outputs t<4 where the reference's
zero-padding of *biased* intermediates differs from the composition.

Device kernel per core (4 samples, processed as 2 sample-PAIRS so every
post-matmul op runs at the full 128 partitions):
  - x is loaded per sample as a [128, SW+8] fp16 SUPERTILE (4 compute
    tiles worth) with tap-pair packing (partitions 0-63 = x shifted 8,
    64-127 = x shifted 7; the second sample of a pair uses the SWAPPED
    layout so its data sits in partitions 64-127 wherever the pair-packed
    elementwise ops need it),
  - the K=7 conv is 4 fp16 matmuls of contraction 128 per 512-col chunk;
    outputs are split by weight columns into a "log_s" PSUM tile and a "t"
    PSUM tile, each [128, TS] holding BOTH samples of the pair (64-col
    matmuls auto-col-tile into PE column groups 0/64, which run
    concurrently),
  - ScalarE: TH = tanh(ps_ls + b_ls) [128,TS]; E = exp(TH*alpha+beta) fp16,
  - VectorE: ZM halves = E * x (fp16, 2x mode); then one fused
    scalar_tensor_tensor: zstage = (ps_t + b_t) + ZM,
  - z stored as fp16 (upcast to fp32 on host; well within tolerance) from
    [128, 2*TS] staging chunks as soon as both halves are written.

DMA plan: x supertile loads are ~0.5 MB transfers (8 KB per-partition
lines) -- x2a halves on sync (HWDGE), x2b halves + z stores on gpsimd
(SWDGE); constants are merged into 3 DMAs; supertile-0 zero halos are
memset, and the first pair's x tiles are issued right after the weights so
the first matmul starts early. PE warm-up matmuls run during the head's
DMA wait to flip the HAM clock gate to 8/8 before real work.
"""

import numpy as np

import concourse.bacc as bacc
import concourse.bass as bass
import concourse.mybir as mybir
import concourse.tile as tile
from concourse.bass_utils import run_bass_kernel_spmd

N_CORES = 8
B, C, T = 32, 64, 8192
NS = B // N_CORES          # samples per core
TS = 1024                  # time-tile width (multiple of 512)
SUP = 4                    # compute tiles per DMA supertile
O = 128                    # output channels (2C)

F32 = mybir.dt.float32
F16 = mybir.dt.float16


# ---------------------------------------------------------------- host math

def _compose(w_in, b_in, w_mid, b_mid, w_out, b_out):
    """W_eff (128, 64, 7), b_eff (128,), D (128, 4)."""
    w_in = np.asarray(w_in, np.float64)
    w_mid = np.asarray(w_mid, np.float64)
    w_out = np.asarray(w_out, np.float64)
    b_in = np.asarray(b_in, np.float64)
    b_mid = np.asarray(b_mid, np.float64)
    b_out = np.asarray(b_out, np.float64)
    H = w_in.shape[0]

    w12 = np.zeros((w_mid.shape[0], w_in.shape[1], 5))
    for i in range(3):
        for j in range(3):
            w12[:, :, i + j] += w_mid[:, :, j] @ w_in[:, :, i]
    w_eff = np.zeros((w_out.shape[0], w_in.shape[1], 7))
    for i in range(5):
        for l in range(3):
            w_eff[:, :, i + l] += w_out[:, :, l] @ w12[:, :, i]

    b_eff = w_out.sum(2) @ (w_mid.sum(2) @ b_in + b_mid) + b_out

    # boundary correction: reference chain on x=0, minus steady-state b_eff
    Tz = 12
    rs1 = np.zeros((H, Tz)) + b_in[:, None]
    rs2 = np.zeros((w_mid.shape[0], Tz))
    for t in range(Tz):
        acc = b_mid.copy()
        for j in range(3):
            tau = t - 2 + j
            if tau >= 0:
                acc = acc + w_mid[:, :, j] @ rs1[:, tau]
        rs2[:, t] = acc
    rs3 = np.zeros((w_out.shape[0], Tz))
    for t in range(Tz):
        acc = b_out.copy()
        for l in range(3):
            tau = t - 2 + l
            if tau >= 0:
                acc = acc + w_out[:, :, l] @ rs2[:, tau]
        rs3[:, t] = acc
    D = rs3[:, 0:4] - b_eff[:, None]
    return (w_eff.astype(np.float32), b_eff.astype(np.float32),
            D.astype(np.float32))


def _pack_weights(w_eff):
    """Stationary lhsT tiles as (p, m, o): m 0-3 = normal layout
    (rows 0-63 = tap 2m, 64-127 = tap 2m+1), m 4-7 = swapped halves."""
    WT = np.zeros((8, 128, 128), np.float32)
    for m in range(4):
        WT[m, 0:64, :] = w_eff[:, :, 2 * m].T
        WT[4 + m, 64:128, :] = w_eff[:, :, 2 * m].T
        if 2 * m + 1 <= 6:
            WT[m, 64:128, :] = w_eff[:, :, 2 * m + 1].T
            WT[4 + m, 0:64, :] = w_eff[:, :, 2 * m + 1].T
    return np.ascontiguousarray(WT.transpose(1, 0, 2))


# ------------------------------------------------------------- device build

def build_nc(ns=NS, t_len=T, ts=TS, sup=SUP):
    assert ts % 512 == 0 and t_len % (ts * sup) == 0
    nc = bacc.Bacc("TRN2", target_bir_lowering=False, debug=False,
                   num_devices=N_CORES)
    xs = nc.dram_tensor("xs", (ns, C, t_len), F16, kind="ExternalInput").ap()
    # host-pretransposed to (p, m, o) so the load is a dense 256 KB DMA
    wt = nc.dram_tensor("wt", (128, 8, 128), F16, kind="ExternalInput").ap()
    # cmat rows: 0-127 = D_ls one-hot lhsT cols, 128-255 = D_t, 256-767 = bind
    cmat = nc.dram_tensor("cmat", (4, 768), F16, kind="ExternalInput").ap()
    # cvec cols: 0 = b_ls, 1 = b_t, 2 = alpha, 3 = beta (dup-packed halves)
    cvec = nc.dram_tensor("cvec", (128, 4), F32, kind="ExternalInput").ap()
    z = nc.dram_tensor("z", (ns, C, t_len), F16, kind="ExternalOutput").ap()

    SW = sup * ts              # supertile span in time
    W4 = SW + 8                # supertile width incl halo
    n_sup = t_len // SW        # supertiles per sample pair
    pairs = ns // 2
    Tanh = mybir.ActivationFunctionType.Tanh
    Exp = mybir.ActivationFunctionType.Exp
    ADD = mybir.AluOpType.add
    MUL = mybir.AluOpType.mult

    with tile.TileContext(nc) as tc:
        with (
            tc.tile_pool(name="consts", bufs=1) as consts,
            tc.tile_pool(name="data", bufs=2) as data,
            tc.tile_pool(name="outs", bufs=3) as outs,
            tc.tile_pool(name="psum", bufs=2, space="PSUM") as psum_pool,
        ):
            def load_xsuper(x2, s, sg, swapped, dma):
                # lo rows get x shifted 8, hi rows x shifted 7; a swapped
                # tile exchanges which partition half holds which shift.
                lo = x2[64:128, :] if swapped else x2[0:64, :]
                hi = x2[0:64, :] if swapped else x2[64:128, :]
                t0 = sg * SW
                if sg == 0:
                    nc.gpsimd.memset(lo[:, 0:8], 0.0)
                    nc.gpsimd.memset(hi[:, 0:7], 0.0)
                    dma(lo[:, 8:W4], xs[s, :, 0:SW])
                    dma(hi[:, 7:W4 - 1], xs[s, :, 0:SW])
                else:
                    dma(lo[:, 0:W4], xs[s, :, t0 - 8:t0 + SW])
                    dma(hi[:, 0:W4 - 1], xs[s, :, t0 - 7:t0 + SW])

            def make_x2(pr, sg):
                x2a = data.tile([128, W4], F16, tag="x2a", name="x2a")
                x2b = data.tile([128, W4], F16, tag="x2b", name="x2b")
                load_xsuper(x2a, 2 * pr, sg, False, nc.sync.dma_start)
                load_xsuper(x2b, 2 * pr + 1, sg, True, nc.gpsimd.dma_start)
                return x2a, x2b

            # weights first (the first matmul's only const dependency),
            # then the first pair of x supertiles, on both DMA dispatchers
            wt_sb = consts.tile([128, 8, 128], F16)
            nc.sync.dma_start(wt_sb, wt)
            cm_sb = consts.tile([4, 768], F16)
            nc.gpsimd.dma_start(cm_sb, cmat)
            pre = [make_x2(0, 0)]
            cv_sb = consts.tile([128, 4], F32)
            nc.sync.dma_start(cv_sb, cvec)
            pre.append(make_x2(0, 1))

            # PE warm-up: ~4us of throwaway matmuls during the head's
            # DMA wait flips the HAM clock gate to 8/8 before real work
            warm = data.tile([128, 512], F16, tag="warm", name="warm",
                             bufs=1)
            nc.vector.memset(warm, 0.0)
            warm_ps = psum_pool.tile([128, 512], F32, tag="psls",
                                     name="warm_ps")
            for _ in range(18):
                nc.tensor.matmul(warm_ps[:, 0:512], warm[:, 0:128],
                                 warm[:, 0:512], start=True, stop=True,
                                 skip_group_check=True)
            bls_sb = cv_sb[:, 0:1]
            btt_sb = cv_sb[:, 1:2]
            avec_sb = cv_sb[:, 2:3]
            bevec_sb = cv_sb[:, 3:4]
            dls_sb = cm_sb[:, 0:128]
            dtt_sb = cm_sb[:, 128:256]
            bind_sb = cm_sb[:, 256:768]

            supers = [(pr, sg) for pr in range(pairs) for sg in range(n_sup)]
            xq = dict(enumerate(pre))
            for k, (pr, sg) in enumerate(supers):
                # prefetch the next supertile's x while this one computes
                kf = k + 1
                if kf < len(supers) and kf not in xq:
                    prf, sgf = supers[kf]
                    xq[kf] = make_x2(prf, sgf)
                x2a, x2b = xq.pop(k)
                s0, s1 = 2 * pr, 2 * pr + 1

                zst = None
                for q in range(sup):
                    it = sg * sup + q      # tile index within the pair
                    o = q * ts             # column offset in the supertile
                    t0 = it * ts

                    ps_ls = psum_pool.tile([128, ts], F32, tag="psls",
                                           name="ps_ls")
                    ps_t = psum_pool.tile([128, ts], F32, tag="pst",
                                          name="ps_t")
                    # conv matmuls: 64-col groups 0 / 64 run concurrently
                    for (ps, o0) in ((ps_ls, 0), (ps_t, 64)):
                        corr = (it == 0)
                        for m in range(4):
                            for h in range(ts // 512):
                                ho = h * 512
                                c0 = o + ho + 2 * m + 1
                                stop = (m == 3) and not (corr and h == 0)
                                nc.tensor.matmul(
                                    ps[0:64, ho:ho + 512],
                                    wt_sb[:, m, o0:o0 + 64],
                                    x2a[:, c0:c0 + 512],
                                    start=(m == 0), stop=stop)
                                nc.tensor.matmul(
                                    ps[64:128, ho:ho + 512],
                                    wt_sb[:, 4 + m, o0:o0 + 64],
                                    x2b[:, c0:c0 + 512],
                                    start=(m == 0), stop=stop)
                        if corr:
                            nc.tensor.matmul(
                                ps[:, 0:512],
                                dls_sb if o0 == 0 else dtt_sb,
                                bind_sb, start=False, stop=True,
                                skip_group_check=True)

                    th = outs.tile([128, ts], F16, tag="th", name="th")
                    nc.scalar.activation(th, ps_ls, Tanh, bias=bls_sb)
                    e = outs.tile([128, ts], F16, tag="e", name="e")
                    nc.scalar.activation(e, th, Exp,
                                         bias=bevec_sb, scale=avec_sb)
                    zm = outs.tile([128, ts], F16, tag="zm", name="zm")
                    nc.vector.tensor_tensor(
                        zm[0:64, :], e[0:64, :],
                        x2a[0:64, 8 + o:8 + o + ts], MUL)
                    nc.vector.tensor_tensor(
                        zm[64:128, :], e[64:128, :],
                        x2b[64:128, 8 + o:8 + o + ts], MUL)
                    # fused: zstage = (ps_t + b_t) + zm  (one DVE op)
                    if q % 2 == 0:
                        zst = outs.tile([128, 2 * ts], F16, tag="zst",
                                        name="zst")
                    zo = (q % 2) * ts
                    nc.vector.scalar_tensor_tensor(
                        zst[:, zo:zo + ts], ps_t, btt_sb, zm, ADD, ADD)
                    if q % 2 == 1:
                        tst = t0 - ts
                        nc.gpsimd.dma_start(z[s0, :, tst:tst + 2 * ts],
                                            zst[0:64, :])
                        nc.gpsimd.dma_start(z[s1, :, tst:tst + 2 * ts],
                                            zst[64:128, :])

    nc.compile()
    return nc


def make_in_maps(x, w_in, b_in, w_mid, b_mid, w_out, b_out, alpha, beta,
                 n_cores=N_CORES):
    w_eff, b_eff, D = _compose(w_in, b_in, w_mid, b_mid, w_out, b_out)
    WT = _pack_weights(w_eff)
    CMAT = np.zeros((4, 768), np.float32)
    CMAT[:, 0:64] = D[0:64, :].T
    CMAT[:, 64:128] = D[0:64, :].T
    CMAT[:, 128:192] = D[64:128, :].T
    CMAT[:, 192:256] = D[64:128, :].T
    for i in range(4):
        CMAT[i, 256 + i] = 1.0
    al = np.asarray(alpha, np.float32).reshape(64)
    be = np.asarray(beta, np.float32).reshape(64)
    CVEC = np.stack([
        np.concatenate([b_eff[0:64], b_eff[0:64]]),
        np.concatenate([b_eff[64:128], b_eff[64:128]]),
        np.concatenate([al, al]),
        np.concatenate([be, be]),
    ], axis=1).astype(np.float32)
    x16 = np.ascontiguousarray(np.asarray(x, np.float32)).astype(np.float16)
    ns = x16.shape[0] // n_cores
    maps = []
    for i in range(n_cores):
        m = dict(xs=np.ascontiguousarray(x16[i * ns:(i + 1) * ns]),
                 wt=WT.astype(np.float16),
                 cmat=CMAT.astype(np.float16),
                 cvec=np.ascontiguousarray(CVEC))
        maps.append(m)
    return maps


_NC_CACHE = {}


def _get_nc():
    if "nc" not in _NC_CACHE:
        _NC_CACHE["nc"] = build_nc()
    return _NC_CACHE["nc"]


def kernel(x, w_in, b_in, w_mid, b_mid, w_out, b_out, alpha, beta,
           _trace=False, _trace_kwargs=None):
    nc = _get_nc()
    in_maps = make_in_maps(x, w_in, b_in, w_mid, b_mid, w_out, b_out,
                           alpha, beta)
    res = run_bass_kernel_spmd(nc, in_maps, core_ids=list(range(N_CORES)),
                               trace=_trace, **(_trace_kwargs or {}))
    out = np.concatenate([r["z"] for r in res.results], axis=0)
    kernel.last_results = res
    return out.astype(np.float32)


# revision 7
# speedup vs baseline: 1.1395x; 1.0568x over previous
"""Fused ARFlow kernel for Trainium2 (8 NeuronCores, data-parallel over batch).

Reference computes three causal K=3 convs (64->256->256->128 ch) with NO
nonlinearity between them, then z = exp(alpha*tanh(ls)+beta)*x + tt.
The convs are linear, so they compose on the host into a single causal K=7
conv (64->128 ch) with an effective bias, exact for t>=4; an x-independent
(weights-only) correction D fixes outputs t<4 where the reference's
zero-padding of *biased* intermediates differs from the composition.

Device kernel per core (4 samples, processed as 2 sample-PAIRS so every
post-matmul op runs at the full 128 partitions):
  - x is loaded per sample as a [128, SW+8] fp16 SUPERTILE (4 compute
    tiles worth) with tap-pair packing (partitions 0-63 = x shifted 8,
    64-127 = x shifted 7; the second sample of a pair uses the SWAPPED
    layout so its data sits in partitions 64-127 wherever the pair-packed
    elementwise ops need it),
  - the K=7 conv is 4 fp16 matmuls of contraction 128 per 512-col chunk;
    outputs are split by weight columns into a "log_s" PSUM tile and a "t"
    PSUM tile, each [128, TS] holding BOTH samples of the pair (64-col
    matmuls auto-col-tile into PE column groups 0/64, which run
    concurrently),
  - ScalarE: TH = tanh(ps_ls + b_ls) [128,TS]; E = exp(TH*alpha+beta) fp16,
  - VectorE: ZM halves = E * x (fp16, 2x mode); then one fused
    scalar_tensor_tensor: zstage = (ps_t + b_t) + ZM,
  - z stored as fp16 (upcast to fp32 on host; well within tolerance) from
    [128, 2*TS] staging chunks as soon as both halves are written.

DMA plan: x supertile loads are ~0.5 MB transfers (8 KB per-partition
lines) -- x2a halves on sync (HWDGE), x2b halves + z stores on gpsimd
(SWDGE); constants are merged into 3 DMAs; supertile-0 zero halos are
memset, and the first pair's x tiles are issued right after the weights so
the first matmul starts early. PE warm-up matmuls run during the head's
DMA wait to flip the HAM clock gate to 8/8 before real work.
"""

import numpy as np

import concourse.bacc as bacc
import concourse.bass as bass
import concourse.mybir as mybir
import concourse.tile as tile
from concourse.bass_utils import run_bass_kernel_spmd

N_CORES = 8
B, C, T = 32, 64, 8192
NS = B // N_CORES          # samples per core
TS = 1024                  # time-tile width (multiple of 512)
SUP = 4                    # compute tiles per DMA supertile
O = 128                    # output channels (2C)

F32 = mybir.dt.float32
F16 = mybir.dt.float16


# ---------------------------------------------------------------- host math

def _compose(w_in, b_in, w_mid, b_mid, w_out, b_out):
    """W_eff (128, 64, 7), b_eff (128,), D (128, 4)."""
    w_in = np.asarray(w_in, np.float64)
    w_mid = np.asarray(w_mid, np.float64)
    w_out = np.asarray(w_out, np.float64)
    b_in = np.asarray(b_in, np.float64)
    b_mid = np.asarray(b_mid, np.float64)
    b_out = np.asarray(b_out, np.float64)
    H = w_in.shape[0]

    w12 = np.zeros((w_mid.shape[0], w_in.shape[1], 5))
    for i in range(3):
        for j in range(3):
            w12[:, :, i + j] += w_mid[:, :, j] @ w_in[:, :, i]
    w_eff = np.zeros((w_out.shape[0], w_in.shape[1], 7))
    for i in range(5):
        for l in range(3):
            w_eff[:, :, i + l] += w_out[:, :, l] @ w12[:, :, i]

    b_eff = w_out.sum(2) @ (w_mid.sum(2) @ b_in + b_mid) + b_out

    # boundary correction: reference chain on x=0, minus steady-state b_eff
    Tz = 12
    rs1 = np.zeros((H, Tz)) + b_in[:, None]
    rs2 = np.zeros((w_mid.shape[0], Tz))
    for t in range(Tz):
        acc = b_mid.copy()
        for j in range(3):
            tau = t - 2 + j
            if tau >= 0:
                acc = acc + w_mid[:, :, j] @ rs1[:, tau]
        rs2[:, t] = acc
    rs3 = np.zeros((w_out.shape[0], Tz))
    for t in range(Tz):
        acc = b_out.copy()
        for l in range(3):
            tau = t - 2 + l
            if tau >= 0:
                acc = acc + w_out[:, :, l] @ rs2[:, tau]
        rs3[:, t] = acc
    D = rs3[:, 0:4] - b_eff[:, None]
    return (w_eff.astype(np.float32), b_eff.astype(np.float32),
            D.astype(np.float32))


def _pack_weights(w_eff):
    """Stationary lhsT tiles as (p, m, o): m 0-3 = normal layout
    (rows 0-63 = tap 2m, 64-127 = tap 2m+1), m 4-7 = swapped halves."""
    WT = np.zeros((8, 128, 128), np.float32)
    for m in range(4):
        WT[m, 0:64, :] = w_eff[:, :, 2 * m].T
        WT[4 + m, 64:128, :] = w_eff[:, :, 2 * m].T
        if 2 * m + 1 <= 6:
            WT[m, 64:128, :] = w_eff[:, :, 2 * m + 1].T
            WT[4 + m, 0:64, :] = w_eff[:, :, 2 * m + 1].T
    return np.ascontiguousarray(WT.transpose(1, 0, 2))


# ------------------------------------------------------------- device build

def build_nc(ns=NS, t_len=T, ts=TS, sup=SUP):
    assert ts % 512 == 0 and t_len % (ts * sup) == 0
    # supertile plan per pair: small leading supertiles so the first
    # matmul's x DMA is small, larger ones in steady state
    first_plan = [1, 1, 2] + [sup] * ((t_len // ts - 4) // sup)
    rest_plan = [sup] * (t_len // ts // sup)
    assert sum(first_plan) == sum(rest_plan) == t_len // ts
    nc = bacc.Bacc("TRN2", target_bir_lowering=False, debug=False,
                   num_devices=N_CORES)
    xs = nc.dram_tensor("xs", (ns, C, t_len), F16, kind="ExternalInput").ap()
    # host-pretransposed to (p, m, o) so the load is a dense 256 KB DMA
    wt = nc.dram_tensor("wt", (128, 8, 128), F16, kind="ExternalInput").ap()
    # cmat rows: 0-127 = D_ls one-hot lhsT cols, 128-255 = D_t, 256-767 = bind
    cmat = nc.dram_tensor("cmat", (4, 768), F16, kind="ExternalInput").ap()
    # cvec cols: 0 = b_ls, 1 = b_t, 2 = alpha, 3 = beta (dup-packed halves)
    cvec = nc.dram_tensor("cvec", (128, 4), F32, kind="ExternalInput").ap()
    z = nc.dram_tensor("z", (ns, C, t_len), F16, kind="ExternalOutput").ap()

    W4 = sup * ts + 8          # max supertile width incl halo
    pairs = ns // 2
    Tanh = mybir.ActivationFunctionType.Tanh
    Exp = mybir.ActivationFunctionType.Exp
    ADD = mybir.AluOpType.add
    MUL = mybir.AluOpType.mult

    with tile.TileContext(nc) as tc:
        with (
            tc.tile_pool(name="consts", bufs=1) as consts,
            tc.tile_pool(name="data", bufs=2) as data,
            tc.tile_pool(name="outs", bufs=3) as outs,
            tc.tile_pool(name="psum", bufs=2, space="PSUM") as psum_pool,
        ):
            def load_xsuper(x2, s, t0, nsub, swapped, dma):
                # lo rows get x shifted 8, hi rows x shifted 7; a swapped
                # tile exchanges which partition half holds which shift.
                lo = x2[64:128, :] if swapped else x2[0:64, :]
                hi = x2[0:64, :] if swapped else x2[64:128, :]
                w = nsub * ts + 8
                if t0 == 0:
                    nc.gpsimd.memset(lo[:, 0:8], 0.0)
                    nc.gpsimd.memset(hi[:, 0:7], 0.0)
                    dma(lo[:, 8:w], xs[s, :, 0:w - 8])
                    dma(hi[:, 7:w - 1], xs[s, :, 0:w - 8])
                else:
                    dma(lo[:, 0:w], xs[s, :, t0 - 8:t0 + nsub * ts])
                    dma(hi[:, 0:w - 1], xs[s, :, t0 - 7:t0 + nsub * ts])

            def make_x2(pr, t0, nsub):
                x2a = data.tile([128, W4], F16, tag="x2a", name="x2a",
                                bufs=3)
                x2b = data.tile([128, W4], F16, tag="x2b", name="x2b",
                                bufs=3)
                load_xsuper(x2a, 2 * pr, t0, nsub, False, nc.sync.dma_start)
                load_xsuper(x2b, 2 * pr + 1, t0, nsub, True,
                            nc.gpsimd.dma_start)
                return x2a, x2b

            # supertile schedule: (pair, t0, nsub)
            supers = []
            for pr in range(pairs):
                plan = first_plan if pr == 0 else rest_plan
                t0 = 0
                for nsub in plan:
                    supers.append((pr, t0, nsub))
                    t0 += nsub * ts

            # weights first (the first matmul's only const dependency),
            # then the first supertiles, on both DMA dispatchers
            wt_sb = consts.tile([128, 8, 128], F16)
            nc.sync.dma_start(wt_sb, wt)
            cm_sb = consts.tile([4, 768], F16)
            nc.gpsimd.dma_start(cm_sb, cmat)
            pre = [make_x2(*supers[0])]
            cv_sb = consts.tile([128, 4], F32)
            nc.sync.dma_start(cv_sb, cvec)
            pre.append(make_x2(*supers[1]))

            # PE warm-up: ~4.5us of throwaway matmuls during the head's
            # DMA wait flips the HAM clock gate to 8/8 before real work
            warm = data.tile([128, 512], F16, tag="warm", name="warm",
                             bufs=1)
            nc.vector.memset(warm, 0.0)
            warm_ps = psum_pool.tile([128, 512], F32, tag="psls",
                                     name="warm_ps")
            for _ in range(9):
                nc.tensor.matmul(warm_ps[:, 0:512], warm[:, 0:128],
                                 warm[:, 0:512], start=True, stop=True,
                                 skip_group_check=True)
            bls_sb = cv_sb[:, 0:1]
            btt_sb = cv_sb[:, 1:2]
            avec_sb = cv_sb[:, 2:3]
            bevec_sb = cv_sb[:, 3:4]
            dls_sb = cm_sb[:, 0:128]
            dtt_sb = cm_sb[:, 128:256]
            bind_sb = cm_sb[:, 256:768]

            xq = dict(enumerate(pre))
            zst = None
            for k, (pr, st0, nsub) in enumerate(supers):
                # prefetch upcoming supertiles' x while this one computes
                for kf in (k + 1, k + 2):
                    if kf < len(supers) and kf not in xq:
                        xq[kf] = make_x2(*supers[kf])
                x2a, x2b = xq.pop(k)
                s0, s1 = 2 * pr, 2 * pr + 1

                for q in range(nsub):
                    it = st0 // ts + q     # tile index within the pair
                    o = q * ts             # column offset in the supertile
                    t0 = it * ts

                    ps_ls = psum_pool.tile([128, ts], F32, tag="psls",
                                           name="ps_ls")
                    ps_t = psum_pool.tile([128, ts], F32, tag="pst",
                                          name="ps_t")
                    # conv matmuls: 64-col groups 0 / 64 run concurrently
                    for (ps, o0) in ((ps_ls, 0), (ps_t, 64)):
                        corr = (it == 0)
                        for m in range(4):
                            for h in range(ts // 512):
                                ho = h * 512
                                c0 = o + ho + 2 * m + 1
                                stop = (m == 3) and not (corr and h == 0)
                                nc.tensor.matmul(
                                    ps[0:64, ho:ho + 512],
                                    wt_sb[:, m, o0:o0 + 64],
                                    x2a[:, c0:c0 + 512],
                                    start=(m == 0), stop=stop)
                                nc.tensor.matmul(
                                    ps[64:128, ho:ho + 512],
                                    wt_sb[:, 4 + m, o0:o0 + 64],
                                    x2b[:, c0:c0 + 512],
                                    start=(m == 0), stop=stop)
                        if corr:
                            nc.tensor.matmul(
                                ps[:, 0:512],
                                dls_sb if o0 == 0 else dtt_sb,
                                bind_sb, start=False, stop=True,
                                skip_group_check=True)

                    th = outs.tile([128, ts], F16, tag="th", name="th")
                    nc.scalar.activation(th, ps_ls, Tanh, bias=bls_sb)
                    e = outs.tile([128, ts], F16, tag="e", name="e")
                    nc.scalar.activation(e, th, Exp,
                                         bias=bevec_sb, scale=avec_sb)
                    zm = outs.tile([128, ts], F16, tag="zm", name="zm")
                    nc.vector.tensor_tensor(
                        zm[0:64, :], e[0:64, :],
                        x2a[0:64, 8 + o:8 + o + ts], MUL)
                    nc.vector.tensor_tensor(
                        zm[64:128, :], e[64:128, :],
                        x2b[64:128, 8 + o:8 + o + ts], MUL)
                    # fused: zstage = (ps_t + b_t) + zm  (one DVE op)
                    if it % 2 == 0:
                        zst = outs.tile([128, 2 * ts], F16, tag="zst",
                                        name="zst")
                    zo = (it % 2) * ts
                    nc.vector.scalar_tensor_tensor(
                        zst[:, zo:zo + ts], ps_t, btt_sb, zm, ADD, ADD)
                    if it % 2 == 1:
                        tst = t0 - ts
                        nc.sync.dma_start(z[s0, :, tst:tst + 2 * ts],
                                          zst[0:64, :])
                        nc.gpsimd.dma_start(z[s1, :, tst:tst + 2 * ts],
                                            zst[64:128, :])

    nc.compile()
    return nc


def make_in_maps(x, w_in, b_in, w_mid, b_mid, w_out, b_out, alpha, beta,
                 n_cores=N_CORES):
    w_eff, b_eff, D = _compose(w_in, b_in, w_mid, b_mid, w_out, b_out)
    WT = _pack_weights(w_eff)
    CMAT = np.zeros((4, 768), np.float32)
    CMAT[:, 0:64] = D[0:64, :].T
    CMAT[:, 64:128] = D[0:64, :].T
    CMAT[:, 128:192] = D[64:128, :].T
    CMAT[:, 192:256] = D[64:128, :].T
    for i in range(4):
        CMAT[i, 256 + i] = 1.0
    al = np.asarray(alpha, np.float32).reshape(64)
    be = np.asarray(beta, np.float32).reshape(64)
    CVEC = np.stack([
        np.concatenate([b_eff[0:64], b_eff[0:64]]),
        np.concatenate([b_eff[64:128], b_eff[64:128]]),
        np.concatenate([al, al]),
        np.concatenate([be, be]),
    ], axis=1).astype(np.float32)
    x16 = np.ascontiguousarray(np.asarray(x, np.float32)).astype(np.float16)
    ns = x16.shape[0] // n_cores
    maps = []
    for i in range(n_cores):
        m = dict(xs=np.ascontiguousarray(x16[i * ns:(i + 1) * ns]),
                 wt=WT.astype(np.float16),
                 cmat=CMAT.astype(np.float16),
                 cvec=np.ascontiguousarray(CVEC))
        maps.append(m)
    return maps


_NC_CACHE = {}


def _get_nc():
    if "nc" not in _NC_CACHE:
        _NC_CACHE["nc"] = build_nc()
    return _NC_CACHE["nc"]


def kernel(x, w_in, b_in, w_mid, b_mid, w_out, b_out, alpha, beta,
           _trace=False, _trace_kwargs=None):
    nc = _get_nc()
    in_maps = make_in_maps(x, w_in, b_in, w_mid, b_mid, w_out, b_out,
                           alpha, beta)
    res = run_bass_kernel_spmd(nc, in_maps, core_ids=list(range(N_CORES)),
                               trace=_trace, **(_trace_kwargs or {}))
    out = np.concatenate([r["z"] for r in res.results], axis=0)
    kernel.last_results = res
    return out.astype(np.float32)
